# revision 5
# baseline (speedup 1.0000x reference)
"""GroupedCBOW Trainium2 kernel (8-core data-parallel).

Per core (8192 samples):
  - poi embeddings (vocab 1000) gathered via indirect DMA from a
    zero-row-extended fp16 table; masked tokens point at the zero row.
  - hour/weekday (vocab 24/7) pooled via count-matrix matmuls
    (is_equal compare grids on DVE, tiny matmuls on PE).
  - ragged mean over L=10 done as a pairwise add tree on DVE, scaled by
    1/len, then transposed via PE into pooled^T [100, 8192] fp16 tiles.
  - logits = pooled^T.T @ W~ with bias folded via a ones-row, fp16
    matmul with fp32 PSUM accumulate, evicted on DVE+ACT, DMA'd out.
"""

import numpy as np
import sys

sys.path.insert(0, "/opt/trn_rl_repo")

B = 65536
NCORE = 8
BC = B // NCORE  # 8192 samples per core
L = 10
D = 100
VP, VH, VW = 1000, 24, 7
NCLS = VP + VH + VW  # 1031
NG = BC // 128  # 64 groups of 128 samples
NCHUNK = 8  # gather chunks per core
GPC = NG // NCHUNK  # 8 groups per chunk
SPC = GPC * 128  # 1024 samples per chunk

_PROGRAM = None


def _build_program():
    import concourse.bass as bass
    import concourse.tile as tile
    from concourse import bacc, mybir
    from concourse.masks import make_identity
    from contextlib import ExitStack

    f16 = mybir.dt.float16
    f32 = mybir.dt.float32
    i32 = mybir.dt.int32

    nc = bacc.Bacc("TRN2", target_bir_lowering=False, debug=False,
                   num_devices=NCORE)

    # DRAM tensors (per-core shapes)
    xp_d = nc.dram_tensor("xp", [128, NG * L], i32, kind="ExternalInput")
    xhw_d = nc.dram_tensor("xhw", [128, NG * 2 * L], i32, kind="ExternalInput")
    eff_d = nc.dram_tensor("eff", [128, NG], i32, kind="ExternalInput")
    tbl_d = nc.dram_tensor("tbl", [VP + 1, D], f16, kind="ExternalInput")
    eh_d = nc.dram_tensor("eh", [VH, D + 1], f16, kind="ExternalInput")
    ew_d = nc.dram_tensor("ew", [VW, D + 1], f16, kind="ExternalInput")
    wt_d = nc.dram_tensor("wt", [3, D + 1, NCLS], f16, kind="ExternalInput")
    op_d = nc.dram_tensor("out_poi", [BC, VP], f32, kind="ExternalOutput")
    oh_d = nc.dram_tensor("out_hour", [BC, VH], f32, kind="ExternalOutput")
    ow_d = nc.dram_tensor("out_wd", [BC, VW], f32, kind="ExternalOutput")

    AOT = mybir.AluOpType

    with tile.TileContext(nc) as tc, ExitStack() as ctx:
        const = ctx.enter_context(tc.tile_pool(name="const", bufs=1))

        # ---- constant loads -------------------------------------------------
        xp_s = const.tile([128, NG * L], i32)
        nc.sync.dma_start(xp_s[:], xp_d.ap())
        eff_s = const.tile([128, NG], i32)
        nc.sync.dma_start(eff_s[:], eff_d.ap())
        eh_s = const.tile([VH, D + 1], f16)
        nc.sync.dma_start(eh_s[:], eh_d.ap())
        ew_s = const.tile([VW, D + 1], f16)
        nc.sync.dma_start(ew_s[:], ew_d.ap())
        # W~ tiles: [101, 3*1031], cols g*1031+n
        wt_s = const.tile([D + 1, 3 * NCLS], f16)
        nc.sync.dma_start(
            wt_s[:].rearrange("k (g n) -> k g n", g=3),
            wt_d.ap().rearrange("g k n -> k g n"),
        )
        ident = const.tile([128, 128], f16)
        make_identity(nc, ident[:])

        iota_l = const.tile([128, L], i32)
        nc.gpsimd.iota(iota_l[:], [[1, L]], channel_multiplier=0)
        iota_v = const.tile([128, VH], i32)
        nc.gpsimd.iota(iota_v[:], [[1, VH]], channel_multiplier=0)
        iv16 = const.tile([128, VH], f16)
        nc.vector.tensor_copy(iv16[:], iota_v[:])

        # recip = 1/len  (f32 then f16)
        eff_f = const.tile([128, NG], f32)
        nc.vector.tensor_copy(eff_f[:], eff_s[:])
        recip_f = const.tile([128, NG], f32)
        nc.vector.reciprocal(recip_f[:], eff_f[:])
        recip16 = const.tile([128, NG], f16)
        nc.vector.tensor_copy(recip16[:], recip_f[:])

        # ---- device-side masking -------------------------------------------
        # mask[p, t, l] = l < len(sample t*128+p)
        mask_s = const.tile([128, NG * L], i32)
        nc.vector.tensor_tensor(
            out=mask_s[:].rearrange("p (t l) -> p t l", l=L),
            in0=eff_s[:].unsqueeze(2).to_broadcast([128, NG, L]),
            in1=iota_l[:].unsqueeze(1).to_broadcast([128, NG, L]),
            op=AOT.is_gt,
        )
        xpm_s = const.tile([128, NG * L], i32)
        nc.vector.memset(xpm_s[:], VP)  # masked -> zero row
        nc.vector.copy_predicated(xpm_s[:], mask_s[:], xp_s[:])

        with tc.tile_pool(name="xhwtmp", bufs=1) as xhwtmp:
            xhw_s = xhwtmp.tile([128, NG * 2 * L], i32)
            nc.sync.dma_start(xhw_s[:], xhw_d.ap())
            xhw16 = const.tile([128, NG * 2 * L], f16)
            nc.vector.tensor_copy(xhw16[:], xhw_s[:])
        # attr-major: cols [0:640] hour, [640:1280] wd
        xhm = const.tile([128, 2 * NG * L], f16)
        HL = NG * L
        nc.vector.memset(xhm[:, 0:HL], float(VH))
        nc.vector.memset(xhm[:, HL:2 * HL], float(VW))
        nc.vector.copy_predicated(xhm[:, 0:HL], mask_s[:], xhw16[:, 0:HL])
        nc.vector.copy_predicated(xhm[:, HL:2 * HL], mask_s[:], xhw16[:, HL:2 * HL])
        xhm_a = xhm[:].rearrange("p (a t l) -> p a t l", a=2, l=L)

        # ---- hour/wd counts -> C^T [V, 8192] fp16 (scaled by 1/len) --------
        c_hT = const.tile([VH, BC], f16)
        c_wT = const.tile([VW, BC], f16)
        HT = NG // 2  # build counts in 2 halves to bound the grid tile
        with ExitStack() as cctx:
            gpool = cctx.enter_context(tc.tile_pool(name="grids", bufs=1))
            cpool = cctx.enter_context(tc.tile_pool(name="counts", bufs=1))
            cps = cctx.enter_context(
                tc.tile_pool(name="cntps", bufs=2, space="PSUM"))
            for half in range(2):
                t0 = half * HT
                for (V, attr, cT) in ((VH, 0, c_hT), (VW, 1, c_wT)):
                    grid = gpool.tile([128, HT * V * L], f16, tag="grid")
                    gv = grid[:].rearrange("p (t v l) -> p t v l", v=V, l=L)
                    nc.vector.tensor_tensor(
                        out=gv,
                        in0=xhm_a[:, attr, t0:t0 + HT, :]
                        .unsqueeze(2).to_broadcast([128, HT, V, L]),
                        in1=iv16[:, :V].unsqueeze(1).unsqueeze(3)
                        .to_broadcast([128, HT, V, L]),
                        op=AOT.is_equal,
                    )
                    # add tree over l: 10 -> 5 -> 2(+carry) -> 1
                    r1 = cpool.tile([128, HT * V * 5], f16, tag="r1")
                    r1v = r1[:].rearrange("p (t v l) -> p t v l", v=V, l=5)
                    nc.vector.tensor_add(r1v, gv[:, :, :, 0:5], gv[:, :, :, 5:10])
                    r2 = cpool.tile([128, HT * V * 2], f16, tag="r2")
                    r2v = r2[:].rearrange("p (t v l) -> p t v l", v=V, l=2)
                    nc.vector.tensor_add(r2v, r1v[:, :, :, 0:2], r1v[:, :, :, 2:4])
                    cnt = cpool.tile([128, HT * V], f16, tag="cnt")
                    cntv = cnt[:].rearrange("p (t v) -> p t v", v=V)
                    nc.vector.tensor_add(
                        cntv, r2v[:, :, :, 0], r2v[:, :, :, 1])
                    nc.vector.tensor_add(cntv, cntv, r1v[:, :, :, 4])
                    nc.vector.tensor_mul(
                        cntv, cntv,
                        recip16[:, t0:t0 + HT].unsqueeze(2)
                        .to_broadcast([128, HT, V]),
                    )
                    # transpose count groups via PE, 4 groups per psum batch
                    for q in range(HT // 4):
                        ps = cps.tile([V, 512], f32, space="PSUM", tag=f"ps{V}")
                        for j in range(4):
                            t = q * 4 + j
                            nc.tensor.matmul(
                                out=ps[:, j * 128:(j + 1) * 128],
                                lhsT=cntv[:, t, :],
                                rhs=ident[:],
                                start=True, stop=True,
                            )
                        nc.vector.tensor_copy(
                            cT[:, (t0 + q * 4) * 128:(t0 + q * 4) * 128 + 512],
                            ps[:],
                        )

        # ---- pooled^T tiles [101, 8192] fp16 -------------------------------
        pt = [const.tile([D + 1, BC], f16, tag=f"pt{g}", name=f"pt{g}")
              for g in range(3)]

        with tc.tile_pool(name="pthps", bufs=2, space="PSUM") as pthps:
            for (V, cT, emb, g) in ((VH, c_hT, eh_s, 1), (VW, c_wT, ew_s, 2)):
                for q in range(BC // 512):
                    ps = pthps.tile([D + 1, 512], f32, space="PSUM")
                    nc.tensor.matmul(
                        out=ps[:],
                        lhsT=emb[:],
                        rhs=cT[:, q * 512:(q + 1) * 512],
                        start=True, stop=True,
                    )
                    nc.vector.tensor_copy(pt[g][:, q * 512:(q + 1) * 512], ps[:])

        # ---- poi gather + tree reduce + transpose --------------------------
        gatp = ctx.enter_context(tc.tile_pool(name="gat", bufs=2))
        tr1p = ctx.enter_context(tc.tile_pool(name="tr1", bufs=2))
        tr2p = ctx.enter_context(tc.tile_pool(name="tr2", bufs=2))
        p3p = ctx.enter_context(tc.tile_pool(name="p3", bufs=2))
        trps = ctx.enter_context(tc.tile_pool(name="trps", bufs=2, space="PSUM"))

        for c in range(NCHUNK):
            gat = gatp.tile([128, GPC * L * D], f16)
            nc.gpsimd.indirect_dma_start(
                out=gat[:],
                out_offset=None,
                in_=tbl_d.ap(),
                in_offset=_bass().IndirectOffsetOnAxis(
                    ap=xpm_s[:, c * GPC * L:(c + 1) * GPC * L], axis=0),
            )
            gv = gat[:].rearrange("p (g lp x d) -> p g lp x d", g=GPC, lp=5, x=2)
            t1 = tr1p.tile([128, GPC * 5 * D], f16)
            t1v = t1[:].rearrange("p (g l d) -> p g l d", g=GPC, l=5)
            nc.vector.tensor_add(
                t1v, gv[:, :, :, 0, :], gv[:, :, :, 1, :])
            t2 = tr2p.tile([128, GPC * 2 * D], f16)
            t2v = t2[:].rearrange("p (g l d) -> p g l d", g=GPC, l=2)
            nc.vector.tensor_add(t2v, t1v[:, :, 0:2, :], t1v[:, :, 2:4, :])
            p3 = p3p.tile([128, GPC * (D + 1)], f16)
            p3f = p3[:].rearrange("p (g d) -> p g d", g=GPC, d=D + 1)
            p3v = p3f[:, :, 0:D]
            nc.vector.memset(p3f[:, :, D:D + 1], 0.0)
            nc.vector.tensor_add(
                p3v, t2v[:, :, 0, :], t2v[:, :, 1, :])
            nc.vector.tensor_add(p3v, p3v, t1v[:, :, 4, :])
            nc.vector.tensor_mul(
                p3v, p3v,
                recip16[:, c * GPC:(c + 1) * GPC].unsqueeze(2)
                .to_broadcast([128, GPC, D]),
            )
            for q in range(GPC // 4):
                ps = trps.tile([D + 1, 512], f32, space="PSUM")
                for j in range(4):
                    g = q * 4 + j
                    nc.tensor.matmul(
                        out=ps[:, j * 128:(j + 1) * 128],
                        lhsT=p3f[:, g, :],
                        rhs=ident[:],
                        start=True, stop=True,
                    )
                col = c * SPC + q * 512
                nc.vector.tensor_copy(pt[0][:, col:col + 512], ps[:])

        # ---- big matmul + evict + DMA out ----------------------------------
        bigps = ctx.enter_context(tc.tile_pool(name="bigps", bufs=2, space="PSUM"))
        lsp = ctx.enter_context(tc.tile_pool(name="ls", bufs=3))
        hwst = ctx.enter_context(tc.tile_pool(name="hwst", bufs=1))
        hw_stage = hwst.tile([128, NG * (VH + VW)], f32)

        for t in range(NG):
            bp = bigps.tile([128, NCLS], f32, space="PSUM")
            for (n0, nsz) in ((0, 512), (512, 512), (1024, NCLS - 1024)):
                for g in range(3):
                    nc.tensor.matmul(
                        out=bp[:, n0:n0 + nsz],
                        lhsT=pt[g][:, t * 128:(t + 1) * 128],
                        rhs=wt_s[:, g * NCLS + n0:g * NCLS + n0 + nsz],
                        start=(g == 0), stop=(g == 2),
                    )
            ls = lsp.tile([128, VP], f32)
            nc.vector.tensor_copy(ls[:, 0:512], bp[:, 0:512])
            nc.scalar.copy(ls[:, 512:VP], bp[:, 512:VP])
            nc.scalar.copy(
                hw_stage[:, t * (VH + VW):(t + 1) * (VH + VW)], bp[:, VP:NCLS])
            nc.sync.dma_start(op_d.ap()[t * 128:(t + 1) * 128, :], ls[:])

        hw_v = hw_stage[:].rearrange("p (t v) -> p t v", v=VH + VW)
        nc.sync.dma_start(
            oh_d.ap().rearrange("(t p) v -> p t v", p=128), hw_v[:, :, 0:VH])
        nc.sync.dma_start(
            ow_d.ap().rearrange("(t p) v -> p t v", p=128), hw_v[:, :, VH:])

    nc.compile()
    return nc


def _bass():
    import concourse.bass as bass
    return bass


def _get_program():
    global _PROGRAM
    if _PROGRAM is None:
        _PROGRAM = _build_program()
    return _PROGRAM


def _host_prep(x, lens, emb_poi, emb_hour, emb_weekday,
               W_poi, b_poi, W_hour, b_hour, W_weekday, b_weekday):
    """Layout/dtype prep only; all arithmetic happens on device."""
    f16 = np.float16
    # tables
    tbl = np.zeros((VP + 1, D), f16)
    tbl[:VP] = np.asarray(emb_poi, np.float32).astype(f16)
    eh = np.zeros((VH, D + 1), f16)
    eh[:, :D] = np.asarray(emb_hour, np.float32).astype(f16)
    ew = np.ones((VW, D + 1), f16)
    ew[:, :D] = np.asarray(emb_weekday, np.float32).astype(f16)
    # W~ [3, 101, 1031] fp16: cols [W_poi.T | W_hour.T | W_wd.T], ones-row bias
    wt = np.zeros((3, D + 1, NCLS), np.float32)
    wt[:, :D, :VP] = np.asarray(W_poi, np.float32).T.reshape(3, D, VP)
    wt[:, :D, VP:VP + VH] = np.asarray(W_hour, np.float32).T.reshape(3, D, VH)
    wt[:, :D, VP + VH:] = np.asarray(W_weekday, np.float32).T.reshape(3, D, VW)
    wt[2, D, :VP] = np.asarray(b_poi, np.float32)
    wt[2, D, VP:VP + VH] = np.asarray(b_hour, np.float32)
    wt[2, D, VP + VH:] = np.asarray(b_weekday, np.float32)
    wt = wt.astype(f16)

    x = np.asarray(x)
    lens = np.asarray(lens)
    in_maps = []
    for core in range(NCORE):
        s0 = core * BC
        xc = x[s0:s0 + BC]  # [8192, 10, 3] int64
        # [t, p, l] -> [p, t*l]
        xr = xc.reshape(NG, 128, L, 3).astype(np.int32)
        xp = np.ascontiguousarray(
            xr[:, :, :, 0].transpose(1, 0, 2).reshape(128, NG * L))
        xhw = np.ascontiguousarray(
            xr[:, :, :, 1:3].transpose(1, 3, 0, 2).reshape(128, 2 * NG * L))
        eff = np.ascontiguousarray(
            np.minimum(lens[s0:s0 + BC], L).reshape(NG, 128).T.astype(np.int32))
        in_maps.append({
            "xp": xp, "xhw": xhw, "eff": eff,
            "tbl": tbl, "eh": eh, "ew": ew, "wt": wt,
        })
    return in_maps


def run_on_device(in_maps, trace=False, **kw):
    from concourse.bass_utils import run_bass_kernel_spmd
    nc = _get_program()
    return run_bass_kernel_spmd(
        nc, in_maps, core_ids=list(range(NCORE)), trace=trace, **kw)


def kernel(**inputs):
    in_maps = _host_prep(**inputs)
    br = run_on_device(in_maps)
    outs = br.results
    poi = np.concatenate([o["out_poi"] for o in outs], axis=0)
    hour = np.concatenate([o["out_hour"] for o in outs], axis=0)
    wd = np.concatenate([o["out_wd"] for o in outs], axis=0)
    return poi, hour, wd
